# revision 3
# baseline (speedup 1.0000x reference)
"""Trainium2 Bass kernel for nn_CausalGraphReasoning.

Reference computation (n=64 nodes, d=128, h=256):
  causal_matrix[i,j]   = sigmoid(MLP_cd(concat(x_i, x_j)))       masked i!=j
  confounder[i,j,k]    = sigmoid(MLP_cf(concat(x_i, x_j, x_k)))  masked distinct
  modified_features    = x with row `node` replaced by MLP_ip(concat(x_node, v))

Key algebraic restructure: the first-layer matmuls over concatenated features
decompose into per-node projections, e.g.
  MLP_cf layer1(i,j,k) = relu(A[i] + B[j] + C[k] + b1)
with A = x@W1[0:d], B = x@W1[d:2d], C = x@W1[2d:3d].  That turns the O(n^3 *
3d*h) triplet matmul into O(n*d*h) projections plus O(n^3*h) broadcast-add-relu
and an O(n^3*h) dot with W2 — ~100x less FLOPs.

Sharding: the i-axis of the triplet (and pair) grid is split across the 8
cores (8 i-values each).  Each core receives the full node set (for j/k) plus
only its own i-rows, computes its [8,64,64] confounder block and [8,64] causal
rows, and the host concatenates the blocks.  Weights are replicated.

On-device layout: hidden channels on partitions (2 half-tiles of 128), batch
in the free dim.  Per k, th_k = relu(D + C[:,k]) is one fused
tensor_scalar(add,max) / activation(Relu, bias) instruction of [128,512],
split between VectorE and ScalarE.  The 256-channel reduction with cfW2 runs
on TensorE as 128 accumulating fp32r matmuls (N=512) whose lhsT is a sliding
[128,64] window over a zero-padded cfW2 column, so row k of a single
[64,512] PSUM bank receives z_k while the other rows accumulate zeros —
avoiding any single-partition evacuation work.
"""

import os
import sys

for _p in ("/opt/trn_rl_repo", "/root/.axon_site", "/root/.axon_site/_ro/trn_rl_repo",
           "/root/.axon_site/_ro/pypackages"):
    if os.path.isdir(_p) and _p not in sys.path:
        sys.path.append(_p)

import numpy as np

N = 64          # nodes
D = 128         # input dim
H = 256         # hidden dim
NCORES = 8
IB = N // NCORES   # i-rows per core = 8
P = IB * N         # pairs per core = 512

_F32 = None  # mybir.dt.float32, set lazily
_PROGRAM = None  # (nc, names) cache


def _build_program():
    import concourse.bacc as bacc
    import concourse.mybir as mybir
    import concourse.tile as tile

    f32 = mybir.dt.float32
    f32r = mybir.dt.float32r
    AF = mybir.ActivationFunctionType
    OP = mybir.AluOpType

    nc = bacc.Bacc("TRN2", target_bir_lowering=False, debug=False,
                   num_devices=NCORES)

    # ---- DRAM I/O ----------------------------------------------------
    d_xT = nc.dram_tensor("xT", [D, N], f32, kind="ExternalInput")
    d_xTi = nc.dram_tensor("xTi", [D, IB], f32, kind="ExternalInput")
    # 10 stacked [128,128] lhsT blocks: cfA0 cfA1 cfB0 cfB1 cfC0 cfC1
    #                                   cdA0 cdA1 cdB0 cdB1
    d_w1s = nc.dram_tensor("w1s", [D, 10 * 128], f32, kind="ExternalInput")
    # cdW2 k-halves [128,128]x2 then 2 spare cols (unused)
    d_w2s = nc.dram_tensor("w2s", [D, 256], f32r, kind="ExternalInput")
    # sliding-window padded cfW2 halves: zeros except col 64 = cfW2 half
    d_w2p = nc.dram_tensor("w2p", [D, 256], f32r, kind="ExternalInput")
    d_w3 = nc.dram_tensor("w3", [D, 1], f32r, kind="ExternalInput")
    d_wip1 = nc.dram_tensor("wip1", [D, 256], f32, kind="ExternalInput")
    d_wip1b = nc.dram_tensor("wip1b", [1, 256], f32, kind="ExternalInput")
    d_wip2 = nc.dram_tensor("wip2", [D, 256], f32, kind="ExternalInput")
    # bias stack cols: 0 cfb1h0, 1 cfb1h1, 2 cdb1h0, 3 cdb1h1, 4 cdb2,
    #                  5 ipb1h0, 6 ipb1h1, 7 ipb2
    d_bst = nc.dram_tensor("bst", [D, 8], f32, kind="ExternalInput")
    # scalar biases replicated down partitions: col0 cfb2, col1 cdb3
    d_sb = nc.dram_tensor("sb", [D, 2], f32, kind="ExternalInput")
    d_trim = nc.dram_tensor("trim", [N, P], f32, kind="ExternalInput")
    d_pairm = nc.dram_tensor("pairm", [1, P], f32, kind="ExternalInput")
    d_xnodeT = nc.dram_tensor("xnodeT", [D, 1], f32, kind="ExternalInput")
    d_ival = nc.dram_tensor("ival", [1, 1], f32, kind="ExternalInput")

    d_conf = nc.dram_tensor("conf_out", [N, P], f32, kind="ExternalOutput")
    d_caus = nc.dram_tensor("causal_out", [1, P], f32, kind="ExternalOutput")
    d_eff = nc.dram_tensor("eff_out", [D, 1], f32, kind="ExternalOutput")

    def r(ap):  # fp32 -> fp32r view for matmul speed (1 cyc/row at N>=256)
        return ap.bitcast(f32r)

    with tile.TileContext(nc) as tc:
        with (
            tc.tile_pool(name="const", bufs=1) as cpool,
            tc.tile_pool(name="work", bufs=1) as wpool,
            tc.tile_pool(name="th", bufs=10) as thpool,
            tc.tile_pool(name="ps1", bufs=4, space="PSUM") as ps1,
            tc.tile_pool(name="psz", bufs=1, space="PSUM") as psz,
            tc.tile_pool(name="psp", bufs=2, space="PSUM") as psp,
        ):
            # ---- load constants -------------------------------------
            xT = cpool.tile([D, N], f32, name="xT_sb")
            xTi = cpool.tile([D, IB], f32, name="xTi_sb")
            w1s = cpool.tile([D, 10 * 128], f32, name="w1s_sb")
            w2s = cpool.tile([D, 256], f32r, name="w2s_sb")
            w2p = cpool.tile([D, 256], f32r, name="w2p_sb")
            w3 = cpool.tile([D, 1], f32r, name="w3_sb")
            wip1 = cpool.tile([D, 256], f32, name="wip1_sb")
            wip1b = cpool.tile([1, 256], f32, name="wip1b_sb")
            wip2 = cpool.tile([D, 256], f32, name="wip2_sb")
            bst = cpool.tile([D, 8], f32, name="bst_sb")
            sb = cpool.tile([D, 2], f32, name="sb_sb")
            trim = cpool.tile([N, P], f32, name="trim_sb")
            pairm = cpool.tile([1, P], f32, name="pairm_sb")
            xnodeT = cpool.tile([D, 1], f32, name="xnodeT_sb")
            ival = cpool.tile([1, 1], f32, name="ival_sb")
            for sbuf, dram in [(w1s, d_w1s), (xT, d_xT), (xTi, d_xTi),
                               (w2s, d_w2s), (w2p, d_w2p), (w3, d_w3),
                               (wip1, d_wip1), (wip1b, d_wip1b),
                               (wip2, d_wip2), (bst, d_bst), (sb, d_sb),
                               (trim, d_trim), (pairm, d_pairm),
                               (xnodeT, d_xnodeT), (ival, d_ival)]:
                nc.sync.dma_start(sbuf[:], dram[:])

            def w1blk(i):
                return w1s[:, i * 128:(i + 1) * 128]

            # ---- stage 1: per-node projections (channels on partitions)
            # A'my [128, IB] x2 halves (cf, bias folded), B/C [128, N] x2,
            # PA'my [128, IB] x2 (cd, bias folded), PB [128, N] x2.
            Bh, Ch, Ah, PAh, PBh = [], [], [], [], []
            for h in range(2):
                pB = ps1.tile([D, N], f32, name=f"pB{h}", tag="ps1")
                nc.tensor.matmul(pB[:], w1blk(2 + h), xT[:], start=True, stop=True)
                t = wpool.tile([D, N], f32, name=f"Bh{h}")
                nc.scalar.copy(t[:], pB[:])
                Bh.append(t)

                pC = ps1.tile([D, N], f32, name=f"pC{h}", tag="ps1")
                nc.tensor.matmul(pC[:], w1blk(4 + h), xT[:], start=True, stop=True)
                t = wpool.tile([D, N], f32, name=f"Ch{h}")
                nc.scalar.copy(t[:], pC[:])
                Ch.append(t)

                pA = ps1.tile([D, N], f32, name=f"pA{h}", tag="ps1")
                nc.tensor.matmul(pA[:, :IB], w1blk(0 + h), xTi[:], start=True, stop=True)
                t = wpool.tile([D, IB], f32, name=f"Ah{h}")
                nc.scalar.activation(t[:], pA[:, :IB], AF.Identity,
                                     bias=bst[:, 0 + h:1 + h])
                Ah.append(t)

                pPA = ps1.tile([D, N], f32, name=f"pPA{h}", tag="ps1")
                nc.tensor.matmul(pPA[:, :IB], w1blk(6 + h), xTi[:], start=True, stop=True)
                t = wpool.tile([D, IB], f32, name=f"PAh{h}")
                nc.scalar.activation(t[:], pPA[:, :IB], AF.Identity,
                                     bias=bst[:, 2 + h:3 + h])
                PAh.append(t)

                pPB = ps1.tile([D, N], f32, name=f"pPB{h}", tag="ps1")
                nc.tensor.matmul(pPB[:], w1blk(8 + h), xT[:], start=True, stop=True)
                t = wpool.tile([D, N], f32, name=f"PBh{h}")
                nc.scalar.copy(t[:], pPB[:])
                PBh.append(t)

            # ---- stage 2: D = A'[i] + B[j]  (triplet pre-act, no relu)
            #      and PH1 = relu(PA'[i] + PB[j]) (pair layer-1 output)
            Dh, PH1h = [], []
            for h in range(2):
                Dt = wpool.tile([D, P], f32, name=f"Dh{h}")
                Pt = wpool.tile([D, P], f32r, name=f"PH1h{h}")
                for di in range(IB):
                    s = slice(di * N, (di + 1) * N)
                    nc.vector.tensor_scalar(
                        Dt[:, s], Bh[h][:], Ah[h][:, di:di + 1], None, OP.add)
                    nc.vector.tensor_scalar(
                        Pt[:, s], PBh[h][:], PAh[h][:, di:di + 1], 0.0,
                        OP.add, OP.max)
                Dh.append(Dt)
                PH1h.append(Pt)

            # ---- intervention head (tiny; runs early on idle engines) --
            ih = []
            for h in range(2):
                c = slice(h * 128, (h + 1) * 128)
                pIh = ps1.tile([D, N], f32, name=f"pI{h}", tag="ps1")
                nc.tensor.matmul(pIh[:, 0:1], wip1[:, c], xnodeT[:],
                                 start=True, stop=False)
                nc.tensor.matmul(pIh[:, 0:1], wip1b[:, c], ival[:],
                                 start=False, stop=True)
                t = wpool.tile([D, 1], f32, name=f"ih{h}")
                nc.scalar.activation(t[:], pIh[:, 0:1], AF.Relu,
                                     bias=bst[:, 5 + h:6 + h])
                ih.append(t)
            pE = ps1.tile([D, N], f32, name="pE", tag="ps1")
            nc.tensor.matmul(pE[:, 0:1], wip2[:, 0:128], ih[0][:],
                             start=True, stop=False)
            nc.tensor.matmul(pE[:, 0:1], wip2[:, 128:256], ih[1][:],
                             start=False, stop=True)
            eff = wpool.tile([D, 1], f32, name="eff")
            nc.scalar.activation(eff[:], pE[:, 0:1], AF.Identity,
                                 bias=bst[:, 7:8])
            nc.sync.dma_start(d_eff[:], eff[:])

            # ---- pair MLP layers 2+3 --------------------------------
            pP = psp.tile([D, 512], f32, name="pP", tag="psp")
            nc.tensor.matmul(pP[:], w2s[:, 0:128], PH1h[0][:],
                             start=True, stop=False)
            nc.tensor.matmul(pP[:], w2s[:, 128:256], PH1h[1][:],
                             start=False, stop=True)
            ph2 = wpool.tile([D, P], f32r, name="ph2")
            nc.scalar.activation(ph2[:], pP[:], AF.Relu, bias=bst[:, 4:5])
            pCz = psp.tile([D, 512], f32, name="pCz", tag="psp")
            nc.tensor.matmul(pCz[0:1, :], w3[:], ph2[:],
                             start=True, stop=True)
            caus = wpool.tile([1, P], f32, name="caus")
            nc.scalar.activation(caus[:], pCz[0:1, :], AF.Sigmoid,
                                 bias=sb[0:1, 1:2])
            nc.vector.tensor_tensor(caus[:], caus[:], pairm[:], OP.mult)
            nc.sync.dma_start(d_caus[:], caus[:])

            # ---- main triplet loop ----------------------------------
            # th = relu(D + C[:,k]) per half -> accumulate z rows into a
            # single [64,512] PSUM bank via sliding-window lhsT over w2p.
            zps = psz.tile([N, 512], f32, name="zps")
            for k in range(N):
                for h in range(2):
                    idx = 2 * k + h
                    th = thpool.tile([D, P], f32r, name="th", tag="th")
                    if idx % 3 == 1:
                        nc.scalar.activation(th[:], Dh[h][:], AF.Relu,
                                             bias=Ch[h][:, k:k + 1])
                    else:
                        nc.vector.tensor_scalar(
                            th[:], Dh[h][:], Ch[h][:, k:k + 1], 0.0,
                            OP.add, OP.max)
                    lhs = w2p[:, 64 + 128 * h - k: 128 + 128 * h - k]
                    nc.tensor.matmul(zps[:], lhs, th[:],
                                     start=(idx == 0), stop=(idx == 2 * N - 1))

            sig = wpool.tile([N, P], f32, name="sig")
            nc.scalar.activation(sig[:], zps[:], AF.Sigmoid, bias=sb[0:N, 0:1])
            nc.vector.tensor_tensor(sig[:], sig[:], trim[:], OP.mult)
            nc.sync.dma_start(d_conf[:], sig[:])

    nc.compile()
    return nc


def _get_program():
    global _PROGRAM
    if _PROGRAM is None:
        _PROGRAM = _build_program()
    return _PROGRAM


def _prep_inputs(inputs):
    """Host-side sharding/layout prep -> list of 8 per-core input dicts."""
    x = np.ascontiguousarray(np.asarray(inputs["node_features"], np.float32))
    node = int(np.asarray(inputs["intervention_node"]))
    ival = np.asarray(inputs["intervention_value"], np.float32).reshape(1, 1)
    cdW1 = np.asarray(inputs["cdW1"], np.float32)
    cdb1 = np.asarray(inputs["cdb1"], np.float32)
    cdW2 = np.asarray(inputs["cdW2"], np.float32)
    cdb2 = np.asarray(inputs["cdb2"], np.float32)
    cdW3 = np.asarray(inputs["cdW3"], np.float32)
    cdb3 = np.asarray(inputs["cdb3"], np.float32)
    cfW1 = np.asarray(inputs["cfW1"], np.float32)
    cfb1 = np.asarray(inputs["cfb1"], np.float32)
    cfW2 = np.asarray(inputs["cfW2"], np.float32)
    cfb2 = np.asarray(inputs["cfb2"], np.float32)
    ipW1 = np.asarray(inputs["ipW1"], np.float32)
    ipb1 = np.asarray(inputs["ipb1"], np.float32)
    ipW2 = np.asarray(inputs["ipW2"], np.float32)
    ipb2 = np.asarray(inputs["ipb2"], np.float32)

    xT = np.ascontiguousarray(x.T)                       # [D, N]
    w1s = np.concatenate([cfW1[0:128], cfW1[128:256], cfW1[256:384],
                          cdW1[0:128], cdW1[128:256]], axis=1)  # [128, 1280]
    w2s = np.zeros((D, 256), np.float32)
    w2s[:, 0:128] = cdW2[0:128]
    w2s[:, 128:256] = cdW2[128:256]
    w2p = np.zeros((D, 256), np.float32)
    w2p[:, 64] = cfW2[0:128, 0]
    w2p[:, 192] = cfW2[128:256, 0]
    w3 = cdW3.reshape(D, 1)
    wip1 = ipW1[0:128]                                   # [128, 256]
    wip1b = ipW1[128:129]                                # [1, 256]
    wip2 = np.concatenate([ipW2[0:128], ipW2[128:256]], axis=1)  # [128, 256]
    bst = np.zeros((D, 8), np.float32)
    bst[:, 0] = cfb1[0:128]
    bst[:, 1] = cfb1[128:256]
    bst[:, 2] = cdb1[0:128]
    bst[:, 3] = cdb1[128:256]
    bst[:, 4] = cdb2
    bst[:, 5] = ipb1[0:128]
    bst[:, 6] = ipb1[128:256]
    bst[:, 7] = ipb2
    sb = np.zeros((D, 2), np.float32)
    sb[:, 0] = float(cfb2[0])
    sb[:, 1] = float(cdb3[0])
    xnodeT = x[node].reshape(D, 1)

    idx = np.arange(N)
    in_maps = []
    for m in range(NCORES):
        i0 = m * IB
        ii = idx[i0:i0 + IB]
        xTi = np.ascontiguousarray(x[i0:i0 + IB].T)      # [D, IB]
        # trim[k, di*64 + j] = 1 if (i0+di, j, k) pairwise distinct
        i_g = ii[None, :, None]                          # [1, IB, 1]
        j_g = idx[None, None, :]                         # [1, 1, N]
        k_g = idx[:, None, None]                         # [N, 1, 1]
        trim = ((i_g != j_g) & (j_g != k_g) & (i_g != k_g)).astype(np.float32)
        trim = trim.reshape(N, P)
        pairm = (ii[:, None] != idx[None, :]).astype(np.float32).reshape(1, P)
        in_maps.append({
            "xT": xT, "xTi": xTi, "w1s": w1s, "w2s": w2s, "w2p": w2p,
            "w3": w3, "wip1": wip1, "wip1b": wip1b, "wip2": wip2,
            "bst": bst, "sb": sb, "trim": trim, "pairm": pairm,
            "xnodeT": xnodeT, "ival": ival,
        })
    return in_maps, x, node


def _run(inputs, trace=False):
    from concourse.bass_utils import run_bass_kernel_spmd

    nc = _get_program()
    in_maps, x, node = _prep_inputs(inputs)
    res = run_bass_kernel_spmd(nc, in_maps, core_ids=list(range(NCORES)),
                               trace=trace)

    causal = np.zeros((N, N), np.float32)
    conf = np.zeros((N, N, N), np.float32)
    for m in range(NCORES):
        i0 = m * IB
        causal[i0:i0 + IB] = res.results[m]["causal_out"].reshape(IB, N)
        co = res.results[m]["conf_out"]                  # [N(k), P]
        conf[i0:i0 + IB] = co.reshape(N, IB, N).transpose(1, 2, 0)
    modified = x.copy()
    modified[node] = res.results[0]["eff_out"][:, 0]
    return (causal, conf, modified), res


def kernel(**inputs):
    outs, _ = _run(inputs, trace=False)
    return outs


# revision 4
# speedup vs baseline: 1.0878x; 1.0878x over previous
"""Trainium2 Bass kernel for nn_CausalGraphReasoning.

Reference computation (n=64 nodes, d=128, h=256):
  causal_matrix[i,j]   = sigmoid(MLP_cd(concat(x_i, x_j)))       masked i!=j
  confounder[i,j,k]    = sigmoid(MLP_cf(concat(x_i, x_j, x_k)))  masked distinct
  modified_features    = x with row `node` replaced by MLP_ip(concat(x_node, v))

Key algebraic restructure: the first-layer matmuls over concatenated features
decompose into per-node projections, e.g.
  MLP_cf layer1(i,j,k) = relu(A[i] + B[j] + C[k] + b1)
with A = x@W1[0:d], B = x@W1[d:2d], C = x@W1[2d:3d].  That turns the O(n^3 *
3d*h) triplet matmul into O(n*d*h) projections plus O(n^3*h) broadcast-add-relu
and an O(n^3*h) dot with W2 — ~100x less FLOPs.

Sharding: the i-axis of the triplet (and pair) grid is split across the 8
cores (8 i-values each).  Each core receives the full node set (for j/k) plus
only its own i-rows, computes its [8,64,64] confounder block and [8,64] causal
rows, and the host concatenates the blocks.  Weights are replicated.

On-device layout: hidden channels on partitions (2 half-tiles of 128), batch
in the free dim.  Per k, th_k = relu(D + C[:,k]) is one fused
tensor_scalar(add,max) / activation(Relu, bias) instruction of [128,512],
split between VectorE and ScalarE.  The 256-channel reduction with cfW2 runs
on TensorE as 128 accumulating fp32r matmuls (N=512) whose lhsT is a sliding
[128,64] window over a zero-padded cfW2 column, so row k of a single
[64,512] PSUM bank receives z_k while the other rows accumulate zeros —
avoiding any single-partition evacuation work.
"""

import os
import sys

for _p in ("/opt/trn_rl_repo", "/root/.axon_site", "/root/.axon_site/_ro/trn_rl_repo",
           "/root/.axon_site/_ro/pypackages"):
    if os.path.isdir(_p) and _p not in sys.path:
        sys.path.append(_p)

import numpy as np

N = 64          # nodes
D = 128         # input dim
H = 256         # hidden dim
NCORES = 8
IB = N // NCORES   # i-rows per core = 8
P = IB * N         # pairs per core = 512

_F32 = None  # mybir.dt.float32, set lazily
_PROGRAM = None  # (nc, names) cache


def _build_program():
    import concourse.bacc as bacc
    import concourse.mybir as mybir
    import concourse.tile as tile

    f32 = mybir.dt.float32
    f32r = mybir.dt.float32r
    f16 = mybir.dt.float16
    AF = mybir.ActivationFunctionType
    OP = mybir.AluOpType

    nc = bacc.Bacc("TRN2", target_bir_lowering=False, debug=False,
                   num_devices=NCORES)

    # ---- DRAM I/O ----------------------------------------------------
    d_xT = nc.dram_tensor("xT", [D, N], f32, kind="ExternalInput")
    d_xTi = nc.dram_tensor("xTi", [D, IB], f32, kind="ExternalInput")
    # 10 stacked [128,128] lhsT blocks: cfA0 cfA1 cfB0 cfB1 cfC0 cfC1
    #                                   cdA0 cdA1 cdB0 cdB1
    d_w1s = nc.dram_tensor("w1s", [D, 10 * 128], f32, kind="ExternalInput")
    # cdW2 k-halves [128,128]x2 then 2 spare cols (unused)
    d_w2s = nc.dram_tensor("w2s", [D, 256], f32r, kind="ExternalInput")
    # sliding-window padded cfW2 halves (fp16): per-half 192-col region,
    # zeros except col 64 (h0) / col 256 (h1) = cfW2 half
    d_w2p = nc.dram_tensor("w2p", [D, 384], f16, kind="ExternalInput")
    d_w3 = nc.dram_tensor("w3", [D, 1], f32r, kind="ExternalInput")
    d_wip1 = nc.dram_tensor("wip1", [D, 256], f32, kind="ExternalInput")
    d_wip1b = nc.dram_tensor("wip1b", [1, 256], f32, kind="ExternalInput")
    d_wip2 = nc.dram_tensor("wip2", [D, 256], f32, kind="ExternalInput")
    # bias stack cols: 0 cfb1h0, 1 cfb1h1, 2 cdb1h0, 3 cdb1h1, 4 cdb2,
    #                  5 ipb1h0, 6 ipb1h1, 7 ipb2
    d_bst = nc.dram_tensor("bst", [D, 8], f32, kind="ExternalInput")
    # scalar biases replicated down partitions: col0 cfb2, col1 cdb3
    d_sb = nc.dram_tensor("sb", [D, 2], f32, kind="ExternalInput")
    d_trim = nc.dram_tensor("trim", [N, P], f32, kind="ExternalInput")
    d_pairm = nc.dram_tensor("pairm", [1, P], f32, kind="ExternalInput")
    d_xnodeT = nc.dram_tensor("xnodeT", [D, 1], f32, kind="ExternalInput")
    d_ival = nc.dram_tensor("ival", [1, 1], f32, kind="ExternalInput")

    d_conf = nc.dram_tensor("conf_out", [N, P], f32, kind="ExternalOutput")
    d_caus = nc.dram_tensor("causal_out", [1, P], f32, kind="ExternalOutput")
    d_eff = nc.dram_tensor("eff_out", [D, 1], f32, kind="ExternalOutput")

    def r(ap):  # fp32 -> fp32r view for matmul speed (1 cyc/row at N>=256)
        return ap.bitcast(f32r)

    with tile.TileContext(nc) as tc:
        with (
            tc.tile_pool(name="const", bufs=1) as cpool,
            tc.tile_pool(name="work", bufs=1) as wpool,
            tc.tile_pool(name="th", bufs=12) as thpool,
            tc.tile_pool(name="ps1", bufs=4, space="PSUM") as ps1,
            tc.tile_pool(name="psz", bufs=1, space="PSUM") as psz,
            tc.tile_pool(name="psp", bufs=2, space="PSUM") as psp,
        ):
            # ---- load constants -------------------------------------
            xT = cpool.tile([D, N], f32, name="xT_sb")
            xTi = cpool.tile([D, IB], f32, name="xTi_sb")
            w1s = cpool.tile([D, 10 * 128], f32, name="w1s_sb")
            w2s = cpool.tile([D, 256], f32r, name="w2s_sb")
            w2p = cpool.tile([D, 384], f16, name="w2p_sb")
            w3 = cpool.tile([D, 1], f32r, name="w3_sb")
            wip1 = cpool.tile([D, 256], f32, name="wip1_sb")
            wip1b = cpool.tile([1, 256], f32, name="wip1b_sb")
            wip2 = cpool.tile([D, 256], f32, name="wip2_sb")
            bst = cpool.tile([D, 8], f32, name="bst_sb")
            sb = cpool.tile([D, 2], f32, name="sb_sb")
            trim = cpool.tile([N, P], f32, name="trim_sb")
            pairm = cpool.tile([1, P], f32, name="pairm_sb")
            xnodeT = cpool.tile([D, 1], f32, name="xnodeT_sb")
            ival = cpool.tile([1, 1], f32, name="ival_sb")
            for sbuf, dram in [(w1s, d_w1s), (xT, d_xT), (xTi, d_xTi),
                               (w2s, d_w2s), (w2p, d_w2p), (w3, d_w3),
                               (wip1, d_wip1), (wip1b, d_wip1b),
                               (wip2, d_wip2), (bst, d_bst), (sb, d_sb),
                               (trim, d_trim), (pairm, d_pairm),
                               (xnodeT, d_xnodeT), (ival, d_ival)]:
                nc.sync.dma_start(sbuf[:], dram[:])

            def w1blk(i):
                return w1s[:, i * 128:(i + 1) * 128]

            # ---- stage 1: per-node projections (channels on partitions)
            # A'my [128, IB] x2 halves (cf, bias folded), B/C [128, N] x2,
            # PA'my [128, IB] x2 (cd, bias folded), PB [128, N] x2.
            Bh, Ch, Ah, PAh, PBh = [], [], [], [], []
            for h in range(2):
                pB = ps1.tile([D, N], f32, name=f"pB{h}", tag="ps1")
                nc.tensor.matmul(pB[:], w1blk(2 + h), xT[:], start=True, stop=True)
                t = wpool.tile([D, N], f16, name=f"Bh{h}")
                nc.scalar.copy(t[:], pB[:])
                Bh.append(t)

                pC = ps1.tile([D, N], f32, name=f"pC{h}", tag="ps1")
                nc.tensor.matmul(pC[:], w1blk(4 + h), xT[:], start=True, stop=True)
                t = wpool.tile([D, N], f32, name=f"Ch{h}")
                nc.scalar.copy(t[:], pC[:])
                Ch.append(t)

                pA = ps1.tile([D, N], f32, name=f"pA{h}", tag="ps1")
                nc.tensor.matmul(pA[:, :IB], w1blk(0 + h), xTi[:], start=True, stop=True)
                t = wpool.tile([D, IB], f32, name=f"Ah{h}")
                nc.scalar.activation(t[:], pA[:, :IB], AF.Identity,
                                     bias=bst[:, 0 + h:1 + h])
                Ah.append(t)

                pPA = ps1.tile([D, N], f32, name=f"pPA{h}", tag="ps1")
                nc.tensor.matmul(pPA[:, :IB], w1blk(6 + h), xTi[:], start=True, stop=True)
                t = wpool.tile([D, IB], f32, name=f"PAh{h}")
                nc.scalar.activation(t[:], pPA[:, :IB], AF.Identity,
                                     bias=bst[:, 2 + h:3 + h])
                PAh.append(t)

                pPB = ps1.tile([D, N], f32, name=f"pPB{h}", tag="ps1")
                nc.tensor.matmul(pPB[:], w1blk(8 + h), xT[:], start=True, stop=True)
                t = wpool.tile([D, N], f32, name=f"PBh{h}")
                nc.scalar.copy(t[:], pPB[:])
                PBh.append(t)

            # ---- stage 2: D = A'[i] + B[j]  (triplet pre-act, no relu)
            #      and PH1 = relu(PA'[i] + PB[j]) (pair layer-1 output)
            Dh, PH1h = [], []
            for h in range(2):
                Dt = wpool.tile([D, P], f16, name=f"Dh{h}")
                Pt = wpool.tile([D, P], f32r, name=f"PH1h{h}")
                for di in range(IB):
                    s = slice(di * N, (di + 1) * N)
                    nc.vector.tensor_scalar(
                        Dt[:, s], Bh[h][:], Ah[h][:, di:di + 1], None, OP.add)
                    nc.vector.tensor_scalar(
                        Pt[:, s], PBh[h][:], PAh[h][:, di:di + 1], 0.0,
                        OP.add, OP.max)
                Dh.append(Dt)
                PH1h.append(Pt)

            # ---- intervention head (tiny; runs early on idle engines) --
            ih = []
            for h in range(2):
                c = slice(h * 128, (h + 1) * 128)
                pIh = ps1.tile([D, N], f32, name=f"pI{h}", tag="ps1")
                nc.tensor.matmul(pIh[:, 0:1], wip1[:, c], xnodeT[:],
                                 start=True, stop=False)
                nc.tensor.matmul(pIh[:, 0:1], wip1b[:, c], ival[:],
                                 start=False, stop=True)
                t = wpool.tile([D, 1], f32, name=f"ih{h}")
                nc.scalar.activation(t[:], pIh[:, 0:1], AF.Relu,
                                     bias=bst[:, 5 + h:6 + h])
                ih.append(t)
            pE = ps1.tile([D, N], f32, name="pE", tag="ps1")
            nc.tensor.matmul(pE[:, 0:1], wip2[:, 0:128], ih[0][:],
                             start=True, stop=False)
            nc.tensor.matmul(pE[:, 0:1], wip2[:, 128:256], ih[1][:],
                             start=False, stop=True)
            eff = wpool.tile([D, 1], f32, name="eff")
            nc.scalar.activation(eff[:], pE[:, 0:1], AF.Identity,
                                 bias=bst[:, 7:8])
            nc.sync.dma_start(d_eff[:], eff[:])

            # ---- pair MLP layers 2+3 --------------------------------
            pP = psp.tile([D, 512], f32, name="pP", tag="psp")
            nc.tensor.matmul(pP[:], w2s[:, 0:128], PH1h[0][:],
                             start=True, stop=False)
            nc.tensor.matmul(pP[:], w2s[:, 128:256], PH1h[1][:],
                             start=False, stop=True)
            ph2 = wpool.tile([D, P], f32r, name="ph2")
            nc.scalar.activation(ph2[:], pP[:], AF.Relu, bias=bst[:, 4:5])
            pCz = psp.tile([D, 512], f32, name="pCz", tag="psp")
            nc.tensor.matmul(pCz[0:1, :], w3[:], ph2[:],
                             start=True, stop=True)
            caus = wpool.tile([1, P], f32, name="caus")
            nc.scalar.activation(caus[:], pCz[0:1, :], AF.Sigmoid,
                                 bias=sb[0:1, 1:2])
            nc.vector.tensor_tensor(caus[:], caus[:], pairm[:], OP.mult)
            nc.sync.dma_start(d_caus[:], caus[:])

            # ---- main triplet loop ----------------------------------
            # th = relu(D + C[:,k]) per half -> accumulate z rows into a
            # single [64,512] PSUM bank via sliding-window lhsT over w2p.
            zps = psz.tile([D, 512], f32, name="zps")
            for k in range(N):
                for h in range(2):
                    idx = 2 * k + h
                    th = thpool.tile([D, P], f16, name="th", tag="th")
                    if idx % 4 == 3:
                        nc.scalar.activation(th[:], Dh[h][:], AF.Relu,
                                             bias=Ch[h][:, k:k + 1])
                    else:
                        nc.vector.tensor_scalar(
                            th[:], Dh[h][:], Ch[h][:, k:k + 1], 0.0,
                            OP.add, OP.max)
                    lhs = w2p[:, 64 + 192 * h - k: 192 + 192 * h - k]
                    nc.tensor.matmul(zps[:], lhs, th[:],
                                     start=(idx == 0), stop=(idx == 2 * N - 1))

            sig = wpool.tile([N, P], f32, name="sig")
            nc.scalar.activation(sig[:], zps[0:N, :], AF.Sigmoid, bias=sb[0:N, 0:1])
            nc.vector.tensor_tensor(sig[:], sig[:], trim[:], OP.mult)
            nc.sync.dma_start(d_conf[:], sig[:])

    nc.compile()
    return nc


def _get_program():
    global _PROGRAM
    if _PROGRAM is None:
        _PROGRAM = _build_program()
    return _PROGRAM


def _prep_inputs(inputs):
    """Host-side sharding/layout prep -> list of 8 per-core input dicts."""
    x = np.ascontiguousarray(np.asarray(inputs["node_features"], np.float32))
    node = int(np.asarray(inputs["intervention_node"]))
    ival = np.asarray(inputs["intervention_value"], np.float32).reshape(1, 1)
    cdW1 = np.asarray(inputs["cdW1"], np.float32)
    cdb1 = np.asarray(inputs["cdb1"], np.float32)
    cdW2 = np.asarray(inputs["cdW2"], np.float32)
    cdb2 = np.asarray(inputs["cdb2"], np.float32)
    cdW3 = np.asarray(inputs["cdW3"], np.float32)
    cdb3 = np.asarray(inputs["cdb3"], np.float32)
    cfW1 = np.asarray(inputs["cfW1"], np.float32)
    cfb1 = np.asarray(inputs["cfb1"], np.float32)
    cfW2 = np.asarray(inputs["cfW2"], np.float32)
    cfb2 = np.asarray(inputs["cfb2"], np.float32)
    ipW1 = np.asarray(inputs["ipW1"], np.float32)
    ipb1 = np.asarray(inputs["ipb1"], np.float32)
    ipW2 = np.asarray(inputs["ipW2"], np.float32)
    ipb2 = np.asarray(inputs["ipb2"], np.float32)

    xT = np.ascontiguousarray(x.T)                       # [D, N]
    w1s = np.concatenate([cfW1[0:128], cfW1[128:256], cfW1[256:384],
                          cdW1[0:128], cdW1[128:256]], axis=1)  # [128, 1280]
    w2s = np.zeros((D, 256), np.float32)
    w2s[:, 0:128] = cdW2[0:128]
    w2s[:, 128:256] = cdW2[128:256]
    w2p = np.zeros((D, 384), np.float16)
    w2p[:, 64] = cfW2[0:128, 0].astype(np.float16)
    w2p[:, 256] = cfW2[128:256, 0].astype(np.float16)
    w3 = cdW3.reshape(D, 1)
    wip1 = ipW1[0:128]                                   # [128, 256]
    wip1b = ipW1[128:129]                                # [1, 256]
    wip2 = np.concatenate([ipW2[0:128], ipW2[128:256]], axis=1)  # [128, 256]
    bst = np.zeros((D, 8), np.float32)
    bst[:, 0] = cfb1[0:128]
    bst[:, 1] = cfb1[128:256]
    bst[:, 2] = cdb1[0:128]
    bst[:, 3] = cdb1[128:256]
    bst[:, 4] = cdb2
    bst[:, 5] = ipb1[0:128]
    bst[:, 6] = ipb1[128:256]
    bst[:, 7] = ipb2
    sb = np.zeros((D, 2), np.float32)
    sb[:, 0] = float(cfb2[0])
    sb[:, 1] = float(cdb3[0])
    xnodeT = x[node].reshape(D, 1)

    idx = np.arange(N)
    in_maps = []
    for m in range(NCORES):
        i0 = m * IB
        ii = idx[i0:i0 + IB]
        xTi = np.ascontiguousarray(x[i0:i0 + IB].T)      # [D, IB]
        # trim[k, di*64 + j] = 1 if (i0+di, j, k) pairwise distinct
        i_g = ii[None, :, None]                          # [1, IB, 1]
        j_g = idx[None, None, :]                         # [1, 1, N]
        k_g = idx[:, None, None]                         # [N, 1, 1]
        trim = ((i_g != j_g) & (j_g != k_g) & (i_g != k_g)).astype(np.float32)
        trim = trim.reshape(N, P)
        pairm = (ii[:, None] != idx[None, :]).astype(np.float32).reshape(1, P)
        in_maps.append({
            "xT": xT, "xTi": xTi, "w1s": w1s, "w2s": w2s, "w2p": w2p,
            "w3": w3, "wip1": wip1, "wip1b": wip1b, "wip2": wip2,
            "bst": bst, "sb": sb, "trim": trim, "pairm": pairm,
            "xnodeT": xnodeT, "ival": ival,
        })
    return in_maps, x, node


def _run(inputs, trace=False):
    from concourse.bass_utils import run_bass_kernel_spmd

    nc = _get_program()
    in_maps, x, node = _prep_inputs(inputs)
    res = run_bass_kernel_spmd(nc, in_maps, core_ids=list(range(NCORES)),
                               trace=trace)

    causal = np.zeros((N, N), np.float32)
    conf = np.zeros((N, N, N), np.float32)
    for m in range(NCORES):
        i0 = m * IB
        causal[i0:i0 + IB] = res.results[m]["causal_out"].reshape(IB, N)
        co = res.results[m]["conf_out"]                  # [N(k), P]
        conf[i0:i0 + IB] = co.reshape(N, IB, N).transpose(1, 2, 0)
    modified = x.copy()
    modified[node] = res.results[0]["eff_out"][:, 0]
    return (causal, conf, modified), res


def kernel(**inputs):
    outs, _ = _run(inputs, trace=False)
    return outs
